# revision 12
# baseline (speedup 1.0000x reference)
# Distributed Trainium2 kernel for nn_Attn (general attention scores + softmax).
#
# reference:
#   proj   = einsum('tbh,dh->tbd', encoder_outputs, W)
#   scores = einsum('hb,tbh->bt', dec_hidden, proj)
#   out    = softmax(scores, axis=1)
#
# Algebraic rewrite: scores[b,t] = sum_h (W^T @ dec_hidden)[h,b] * enc[t,b,h].
# So precompute q = W^T @ dec_hidden (tiny [H,B] matmul on the PE) and the rest
# is a single streaming pass over encoder_outputs -> pure memory-bound.
#
# Sharding: T is split across the 8 cores (enc shard [T/8, B, H] per core);
# W and dec_hidden are replicated.  Local scores [B, T/8] are AllGathered and
# every core computes the softmax over the full T redundantly.
import sys

for _p in ("/opt/trn_rl_repo", "/opt/pypackages"):
    if _p not in sys.path:
        sys.path.append(_p)

import numpy as np

import concourse.bass as bass
import concourse.mybir as mybir
from concourse.bass_utils import run_bass_kernel_spmd

H = 1024
B = 16
T = 4096
NCORES = 8
T_L = T // NCORES          # 512 timesteps per core
ROWS = T_L * B             # 8192 (t,b) rows per core
NCHUNK = 8                 # streaming chunks per core
CROWS = ROWS // NCHUNK     # 1024 rows per chunk (4 MB)
SUB = CROWS // 128         # 8 [128, H] sub-tiles per chunk
NTILES = NCHUNK * SUB      # 64
RING = 3                   # chunk ring buffers
FP32 = mybir.dt.float32


def build_nc() -> bass.Bass:
    nc = bass.Bass(num_devices=NCORES)

    enc = nc.declare_dram_parameter("enc", [ROWS, H], FP32, isOutput=False)
    dec = nc.declare_dram_parameter("dec", [H, B], FP32, isOutput=False)
    w = nc.declare_dram_parameter("w", [H, H], FP32, isOutput=False)
    out = nc.declare_dram_parameter("out", [B, T], FP32, isOutput=True)

    # collective bounce buffers (collectives can't touch I/O tensors).
    # scores stay in the raw on-chip layout [p=(k b), col=(c j)] where the
    # global t = core*512 + c*64 + j*8 + k; softmax is order-invariant over t,
    # so the permutation is only undone at the very end (free in the final
    # multiply's read AP).
    scores_raw = nc.dram_tensor("scores_raw", [128, NTILES], FP32)
    gathered = nc.dram_tensor("gathered", [NCORES, 128, NTILES], FP32, addr_space="Shared")

    from contextlib import ExitStack

    with ExitStack() as ctx:
        w_sb = ctx.enter_context(nc.sbuf_tensor("w_sb", [128, 8 * H], FP32))
        dec_sb = ctx.enter_context(nc.sbuf_tensor("dec_sb", [128, 8 * B], FP32))
        dec_rep = ctx.enter_context(nc.sbuf_tensor("dec_rep", [128, 8 * 128], FP32))
        q_tiled = ctx.enter_context(nc.sbuf_tensor("q_tiled", [128, H], FP32))
        prod = ctx.enter_context(nc.sbuf_tensor("prod", [128, H], FP32))
        scores_buf = ctx.enter_context(nc.sbuf_tensor("scores_buf", [128, NTILES], FP32))
        ring0 = ctx.enter_context(nc.sbuf_tensor("ring0", [128, SUB * H], FP32))
        ring1 = ctx.enter_context(nc.sbuf_tensor("ring1", [128, SUB * H], FP32))
        ring2 = ctx.enter_context(nc.sbuf_tensor("ring2", [128, SUB * H], FP32))
        soft_in = ctx.enter_context(nc.sbuf_tensor("soft_in", [B, T], FP32))
        e_sb = ctx.enter_context(nc.sbuf_tensor("e_sb", [B, T], FP32))
        o_sb = ctx.enter_context(nc.sbuf_tensor("o_sb", [B, T], FP32))
        negmax = ctx.enter_context(nc.sbuf_tensor("negmax", [B, 1], FP32))
        ssum = ctx.enter_context(nc.sbuf_tensor("ssum", [B, 1], FP32))
        rinv = ctx.enter_context(nc.sbuf_tensor("rinv", [B, 1], FP32))
        psum0 = ctx.enter_context(nc.psum_tensor("psum0", [128, 512], FP32))
        psum1 = ctx.enter_context(nc.psum_tensor("psum1", [128, 512], FP32))
        sem_w = ctx.enter_context(nc.semaphore("sem_w"))
        sem_dec = ctx.enter_context(nc.semaphore("sem_dec"))
        sem_s0 = ctx.enter_context(nc.semaphore("sem_s0"))
        sem_s1 = ctx.enter_context(nc.semaphore("sem_s1"))
        sem_s2 = ctx.enter_context(nc.semaphore("sem_s2"))
        sem_scat = ctx.enter_context(nc.semaphore("sem_scat"))
        sem_soft = ctx.enter_context(nc.semaphore("sem_soft"))
        sem_final = ctx.enter_context(nc.semaphore("sem_final"))
        v_prep = ctx.enter_context(nc.semaphore("v_prep"))
        pe_done = ctx.enter_context(nc.semaphore("pe_done"))
        v_done = ctx.enter_context(nc.semaphore("v_done"))
        cc_sem = ctx.enter_context(nc.semaphore("cc_sem"))
        v_soft = ctx.enter_context(nc.semaphore("v_soft"))
        a_soft = ctx.enter_context(nc.semaphore("a_soft"))
        block = ctx.enter_context(nc.Block())
        rings = [ring0, ring1, ring2]
        psums = [psum0, psum1]
        slot_sems = [sem_s0, sem_s1, sem_s2]

        @block.sync
        def _(sync):
            # W: [H, H] -> [128, (dc h)]  (W[dc*128+p, h] at free dc*H+h)
            sync.dma_start(
                out=w_sb[:],
                in_=w[:].rearrange("(dc p) h -> p dc h", p=128),
            ).then_inc(sem_w, 16)
            # dec: [H, B] -> [128, (dc b)]
            sync.dma_start(
                out=dec_sb[:],
                in_=dec[:].rearrange("(dc p) b -> p dc b", p=128),
            ).then_inc(sem_dec, 16)
            # enc chunks: rows [c*CROWS, (c+1)*CROWS) -> [128, (j h)]
            for c in range(NCHUNK):
                if c >= RING:
                    sync.wait_ge(v_done, c - RING + 1)
                sync.dma_start(
                    out=rings[c % RING][:],
                    in_=enc[c * CROWS:(c + 1) * CROWS, :].rearrange(
                        "(j p) h -> p j h", p=128
                    ),
                ).then_inc(slot_sems[c % RING], 16)

        @block.vector
        def _(vector):
            # dec_rep[p, (dc k b)] = dec_sb[p, (dc b)]  (repeat 8x along k)
            vector.wait_ge(sem_dec, 16)
            vector.tensor_copy(
                dec_rep[:].rearrange("p (dc k b) -> p dc k b", dc=8, k=8),
                dec_sb[:]
                .rearrange("p (dc b) -> p dc b", dc=8)
                .unsqueeze(2)
                .broadcast_to([128, 8, 8, B]),
            ).then_inc(v_prep, 1)

            # q_tiled[p, h] = q[h, p%16] from PSUM
            vector.wait_ge(pe_done, 1)
            vector.tensor_copy(q_tiled[:, 0:512], psum0[:])
            vector.tensor_copy(q_tiled[:, 512:1024], psum1[:])
            vector.drain()

            # main streaming loop: fused multiply + free-axis reduce
            for c in range(NCHUNK):
                vector.wait_ge(slot_sems[c % RING], 16 * (c // RING + 1))
                for j in range(SUB):
                    i = c * SUB + j
                    ins = vector.scalar_tensor_tensor(
                        out=rings[c % RING][:, j * H:(j + 1) * H],
                        in0=rings[c % RING][:, j * H:(j + 1) * H],
                        scalar=0.0,
                        in1=q_tiled[:],
                        op0=mybir.AluOpType.add,
                        op1=mybir.AluOpType.mult,
                        accum_out=scores_buf[:, i:i + 1],
                    )
                    if j == SUB - 1:
                        ins.then_inc(v_done, 1)

            # softmax (vector parts)
            vector.wait_ge(sem_soft, 16)
            vector.tensor_reduce(
                negmax[:],
                soft_in[:],
                axis=mybir.AxisListType.X,
                op=mybir.AluOpType.max,
                negate=True,
            ).then_inc(v_soft, 1)
            vector.wait_ge(a_soft, 1)
            vector.reciprocal(rinv[:], ssum[:])
            vector.drain()
            # final normalize; the read AP undoes the t-permutation:
            # e_sb free layout is f=(cc k c j); o_sb is t-ordered (cc c j k)
            vector.tensor_scalar_mul(
                o_sb[:].rearrange("b (cc c j k) -> b cc c j k", cc=8, c=8, j=8),
                e_sb[:].rearrange("b (cc k c j) -> b cc c j k", cc=8, k=8, c=8),
                rinv[:],
            ).then_inc(v_soft, 1)

        @block.tensor
        def _(tensor):
            # q_tiled[p, h] = sum_d dec[d, p%16] * W[d, h]
            tensor.wait_ge(sem_w, 16)
            tensor.wait_ge(v_prep, 1)
            last = None
            for half in range(2):
                for dc in range(8):
                    last = tensor.matmul(
                        psums[half][:],
                        dec_rep[:, dc * 128:(dc + 1) * 128],
                        w_sb[:, dc * H + half * 512: dc * H + half * 512 + 512],
                        start=(dc == 0),
                        stop=(dc == 7),
                    )
            last.then_inc(pe_done, 1)

        @block.scalar
        def _(scalar):
            # raw contiguous dump of the local scores
            scalar.wait_ge(v_done, NCHUNK)
            scalar.dma_start(
                out=scores_raw[:],
                in_=scores_buf[:],
            ).then_inc(sem_scat, 16)
            # gathered[cc, (k b), (c j)] -> soft_in[b, (cc k c j)]
            scalar.wait_ge(cc_sem, 1)
            scalar.dma_start(
                out=soft_in[:].rearrange("b (cc k cj) -> b cc k cj", cc=NCORES, k=SUB),
                in_=gathered[:].rearrange("cc (k b) cj -> b cc k cj", k=SUB),
            ).then_inc(sem_soft, 16)
            # exp(x - max) with fused row-sum
            scalar.wait_ge(v_soft, 1)
            scalar.activation(
                e_sb[:],
                soft_in[:],
                mybir.ActivationFunctionType.Exp,
                bias=negmax[:],
                scale=1.0,
                accum_out=ssum[:],
            ).then_inc(a_soft, 1)
            # store final output
            scalar.wait_ge(v_soft, 2)
            scalar.dma_start(out=out[:], in_=o_sb[:]).then_inc(sem_final, 16)
            scalar.wait_ge(sem_final, 16)

        @block.gpsimd
        def _(gpsimd):
            gpsimd.wait_ge(sem_scat, 16)
            gpsimd.collective_compute(
                "AllGather",
                mybir.AluOpType.bypass,
                replica_groups=[list(range(NCORES))],
                ins=[scores_raw.ap().opt()],
                outs=[gathered.ap().opt()],
            ).then_inc(cc_sem, 1)

    return nc


def make_in_maps(dec_hidden, encoder_outputs, W):
    dec_np = np.ascontiguousarray(np.asarray(dec_hidden, dtype=np.float32))
    enc_np = np.ascontiguousarray(np.asarray(encoder_outputs, dtype=np.float32))
    w_np = np.ascontiguousarray(np.asarray(W, dtype=np.float32))
    assert dec_np.shape == (H, B)
    assert enc_np.shape == (T, B, H)
    assert w_np.shape == (H, H)
    in_maps = []
    for c in range(NCORES):
        shard = np.ascontiguousarray(
            enc_np[c * T_L:(c + 1) * T_L].reshape(ROWS, H)
        )
        in_maps.append({"enc": shard, "dec": dec_np, "w": w_np})
    return in_maps


def _install_ntff_hook():
    """The image's antenv lacks axon_hooks; shim it and register the
    ctypes NTFF profile hook so trace=True works under axon."""
    import types

    if "antenv.axon_hooks" in sys.modules:
        return
    import antenv

    mod = types.ModuleType("antenv.axon_hooks")
    state = {"hook": None}
    mod.set_axon_ntff_profile_hook = lambda h: state.__setitem__("hook", h)
    mod.get_axon_ntff_profile_hook = lambda: state["hook"]
    sys.modules["antenv.axon_hooks"] = mod
    antenv.axon_hooks = mod
    try:
        from trn_agent_boot.trn_boot import _ntff_profile_via_ctypes

        mod.set_axon_ntff_profile_hook(
            _ntff_profile_via_ctypes("/opt/axon/libaxon_pjrt.so")
        )
    except Exception as e:  # degrade to no tracing
        print(f"ntff hook install failed: {e}", file=sys.stderr)


def run(dec_hidden, encoder_outputs, W, trace=False):
    if trace:
        _install_ntff_hook()
    nc = build_nc()
    in_maps = make_in_maps(dec_hidden, encoder_outputs, W)
    res = run_bass_kernel_spmd(
        nc, in_maps, core_ids=list(range(NCORES)), trace=trace
    )
    out = np.asarray(res.results[0]["out"], dtype=np.float32)
    return out, res


def kernel(dec_hidden, encoder_outputs, W):
    out, _ = run(dec_hidden, encoder_outputs, W, trace=False)
    return out


# revision 20
# speedup vs baseline: 1.1737x; 1.1737x over previous
# Distributed Trainium2 kernel for nn_Attn (general attention scores + softmax).
#
# reference:
#   proj   = einsum('tbh,dh->tbd', encoder_outputs, W)
#   scores = einsum('hb,tbh->bt', dec_hidden, proj)
#   out    = softmax(scores, axis=1)
#
# Algebraic rewrite: scores[b,t] = sum_h (W^T @ dec_hidden)[h,b] * enc[t,b,h].
# Precompute q = W^T @ dec_hidden on the PE (tiny), then one streaming pass
# over encoder_outputs with a fused multiply+row-reduce on VectorE -> purely
# HBM-bandwidth-bound.
#
# Sharding: T is split across the 8 cores (enc shard [T/8, B, H] per core);
# W and dec_hidden are replicated.  Local scores [128, 64] tiles are exchanged
# core-to-core with remote_dma_broadcast (SBUF->SBUF over the on-chip fabric,
# ~us) instead of an ncfw AllGather (~40us); a 1-byte collective issued at
# kernel start acts as the entry barrier and completes under the streaming.
# Softmax runs over a t-permuted layout (order-invariant) and the final
# normalize's read AP undoes the permutation.
import sys
from contextlib import ExitStack

for _p in ("/opt/trn_rl_repo", "/opt/pypackages"):
    if _p not in sys.path:
        sys.path.append(_p)

import numpy as np

import concourse.bass as bass
import concourse.bacc as bacc
import concourse.mybir as mybir
from concourse.bass_utils import run_bass_kernel_spmd

H = 1024
B = 16
T = 4096
NCORES = 8
T_L = T // NCORES          # 512 timesteps per core
ROWS = T_L * B             # 8192 (t,b) rows per core
NCHUNK = 8                 # streaming chunks per core
CROWS = ROWS // NCHUNK     # 1024 rows per chunk (4 MB)
SUB = CROWS // 128         # 8 [128, H] sub-tiles per chunk
NTILES = NCHUNK * SUB      # 64
RING = 4                   # chunk ring buffers (slot 3 = w_sb, reused)
FP32 = mybir.dt.float32


def build_nc() -> bass.Bass:
    nc = bacc.Bacc(num_devices=NCORES)

    enc = nc.declare_dram_parameter("enc", [ROWS, H], FP32, isOutput=False)
    dec = nc.declare_dram_parameter("dec", [H, B], FP32, isOutput=False)
    w = nc.declare_dram_parameter("w", [H, H], FP32, isOutput=False)
    out = nc.declare_dram_parameter("out", [B, T], FP32, isOutput=True)

    with ExitStack() as ctx:
        # w_sb doubles as ring slot 3 once q is computed
        w_sb = ctx.enter_context(nc.sbuf_tensor("w_sb", [128, SUB * H], FP32))
        ring0 = ctx.enter_context(nc.sbuf_tensor("ring0", [128, SUB * H], FP32))
        ring1 = ctx.enter_context(nc.sbuf_tensor("ring1", [128, SUB * H], FP32))
        ring2 = ctx.enter_context(nc.sbuf_tensor("ring2", [128, SUB * H], FP32))
        dec_sb = ctx.enter_context(nc.sbuf_tensor("dec_sb", [128, 8 * B], FP32))
        dec_rep = ctx.enter_context(nc.sbuf_tensor("dec_rep", [128, 8 * 128], FP32))
        q_tiled = ctx.enter_context(nc.sbuf_tensor("q_tiled", [128, H], FP32))
        scores_buf = ctx.enter_context(nc.sbuf_tensor("scores_buf", [128, NTILES], FP32))
        gathered_sb = ctx.enter_context(
            nc.sbuf_tensor("gathered_sb", [128, NCORES * NTILES], FP32)
        )
        soft_in = ctx.enter_context(nc.sbuf_tensor("soft_in", [B, T], FP32))
        e_sb = ctx.enter_context(nc.sbuf_tensor("e_sb", [B, T], FP32))
        o_sb = ctx.enter_context(nc.sbuf_tensor("o_sb", [B, T], FP32))
        negmax = ctx.enter_context(nc.sbuf_tensor("negmax", [B, 1], FP32))
        ssum = ctx.enter_context(nc.sbuf_tensor("ssum", [B, 1], FP32))
        rinv = ctx.enter_context(nc.sbuf_tensor("rinv", [B, 1], FP32))
        psum0 = ctx.enter_context(nc.psum_tensor("psum0", [128, 512], FP32))
        psum1 = ctx.enter_context(nc.psum_tensor("psum1", [128, 512], FP32))

        sem_dec = ctx.enter_context(nc.semaphore("sem_dec"))
        wsems = [ctx.enter_context(nc.semaphore(f"sem_w{i}")) for i in range(8)]
        slot_sems = [ctx.enter_context(nc.semaphore(f"sem_s{i}")) for i in range(RING)]
        prep_sem = ctx.enter_context(nc.semaphore("prep_sem"))
        gather_sem = ctx.enter_context(nc.semaphore("gather_sem"))
        lsem_rdma = ctx.enter_context(nc.semaphore("lsem_rdma"))
        soft_sems = [ctx.enter_context(nc.semaphore(f"sem_soft{i}")) for i in range(SUB)]
        sem_final = ctx.enter_context(nc.semaphore("sem_final"))
        v_prep = ctx.enter_context(nc.semaphore("v_prep"))
        pe_done = ctx.enter_context(nc.semaphore("pe_done"))
        v_done = ctx.enter_context(nc.semaphore("v_done"))
        v_soft = ctx.enter_context(nc.semaphore("v_soft"))
        a_soft = ctx.enter_context(nc.semaphore("a_soft"))
        block = ctx.enter_context(nc.Block())

        rings = [ring0, ring1, ring2, w_sb]
        psums = [psum0, psum1]

        @block.sync
        def _(sync):
            # dec first (tiny), then W in 8 pipelined chunks, then enc chunks
            sync.dma_start(
                out=dec_sb[:],
                in_=dec[:].rearrange("(dc p) b -> p dc b", p=128),
            ).then_inc(sem_dec, 16)
            for dc in range(8):
                sync.dma_start(
                    out=w_sb[:, dc * H:(dc + 1) * H],
                    in_=w[dc * 128:(dc + 1) * 128, :],
                ).then_inc(wsems[dc], 16)
            for c in range(NCHUNK):
                if c == RING - 1:
                    # slot 3 is w_sb: wait for the q matmuls to finish with W
                    sync.wait_ge(pe_done, 1)
                if c >= RING:
                    sync.wait_ge(v_done, c - RING + 1)
                sync.dma_start(
                    out=rings[c % RING][:],
                    in_=enc[c * CROWS:(c + 1) * CROWS, :].rearrange(
                        "(j p) h -> p j h", p=128
                    ),
                ).then_inc(slot_sems[c % RING], 16)

        @block.vector
        def _(vector):
            # dec_rep[p, (dc k b)] = dec_sb[p, (dc b)]  (repeat 8x along k)
            vector.wait_ge(sem_dec, 16)
            vector.tensor_copy(
                dec_rep[:].rearrange("p (dc k b) -> p dc k b", dc=8, k=8),
                dec_sb[:]
                .rearrange("p (dc b) -> p dc b", dc=8)
                .unsqueeze(2)
                .broadcast_to([128, 8, 8, B]),
            ).then_inc(v_prep, 1)

            # q_tiled[p, h] = q[h, p%16] from PSUM
            vector.wait_ge(pe_done, 1)
            vector.tensor_copy(q_tiled[:, 0:512], psum0[:])
            vector.tensor_copy(q_tiled[:, 512:1024], psum1[:])
            vector.drain()

            # main streaming loop: fused multiply + free-axis reduce, in-place
            for c in range(NCHUNK):
                vector.wait_ge(slot_sems[c % RING], 16 * (c // RING + 1))
                for j in range(SUB):
                    i = c * SUB + j
                    ins = vector.scalar_tensor_tensor(
                        out=rings[c % RING][:, j * H:(j + 1) * H],
                        in0=rings[c % RING][:, j * H:(j + 1) * H],
                        scalar=0.0,
                        in1=q_tiled[:],
                        op0=mybir.AluOpType.add,
                        op1=mybir.AluOpType.mult,
                        accum_out=scores_buf[:, i:i + 1],
                    )
                    if j == SUB - 1:
                        ins.then_inc(v_done, 1)

            # softmax (vector parts); soft_in free layout f = (k, cc, c, j)
            for k in range(SUB):
                vector.wait_ge(soft_sems[k], 16)
            vector.tensor_reduce(
                negmax[:],
                soft_in[:],
                axis=mybir.AxisListType.X,
                op=mybir.AluOpType.max,
                negate=True,
            ).then_inc(v_soft, 1)
            vector.wait_ge(a_soft, 1)
            vector.reciprocal(rinv[:], ssum[:])
            vector.drain()
            # final normalize; read AP maps t=(cc c j k) -> f=(k cc c j)
            vector.tensor_scalar_mul(
                o_sb[:].rearrange("b (cc c j k) -> b cc c j k", cc=8, c=8, j=8),
                e_sb[:].rearrange("b (k cc c j) -> b cc c j k", k=8, cc=8, c=8),
                rinv[:],
            ).then_inc(v_soft, 1)

        @block.tensor
        def _(tensor):
            # q_tiled[p, h] = sum_d dec[d, p%16] * W[d, h], chunk-pipelined on W
            tensor.wait_ge(v_prep, 1)
            last = None
            for dc in range(8):
                tensor.wait_ge(wsems[dc], 16)
                for half in range(2):
                    last = tensor.matmul(
                        psums[half][:],
                        dec_rep[:, dc * 128:(dc + 1) * 128],
                        w_sb[:, dc * H + half * 512: dc * H + half * 512 + 512],
                        start=(dc == 0),
                        stop=(dc == 7),
                    )
            last.then_inc(pe_done, 1)

        @block.gpsimd
        def _(gpsimd):
            # pre-generate the broadcast descriptors (hides SWDGE latency);
            # slot = my core id (the AP offset is runtime-computed)
            pid = gpsimd.partition_id()
            gpsimd.remote_dma_broadcast(
                out_ap=gathered_sb[:, bass.ts(pid, NTILES)],
                in_ap=scores_buf[:],
                remote_sem=gather_sem,
                local_sem=lsem_rdma,
                rdests=[(0, k) for k in range(NCORES)],
            ).then_inc(prep_sem, 1)
            gpsimd.wait_ge(prep_sem, 1)
            # entry barrier (ncfw prelude AllGather, overlaps the stream phase):
            # remote SBUF writes are only safe once every peer started its NEFF
            gpsimd.bir_kernel_barrier_wait([list(range(NCORES))])
            gpsimd.wait_ge(v_done, NCHUNK)  # local scores complete
            gpsimd.trigger_dma()
            gpsimd.wait_ge(lsem_rdma, 16)  # sends complete

        @block.scalar
        def _(scalar):
            # all 8 cores' tiles arrived: rearrange SBUF->SBUF into row-major
            # per-b layout.  gathered_sb partition p=(k b), free (cc, c, j);
            # soft_in[b, f] with f=(k, cc, c, j).
            scalar.wait_ge(gather_sem, 16)
            # src iterates (k, b; f) partition-major; dst flat addr
            # b*4096 + k*512 + f  ->  soft_in[b, (k cc c j)]
            # partition group k (=t_l%8) -> free block k of soft_in
            for k in range(SUB):
                scalar.dma_start(
                    out=soft_in[:, k * 512:(k + 1) * 512],
                    in_=gathered_sb[k * B:(k + 1) * B, :],
                ).then_inc(soft_sems[k], 16)
            # exp(x - max) with fused row-sum
            scalar.wait_ge(v_soft, 1)
            scalar.activation(
                e_sb[:],
                soft_in[:],
                mybir.ActivationFunctionType.Exp,
                bias=negmax[:],
                scale=1.0,
                accum_out=ssum[:],
            ).then_inc(a_soft, 1)
            # store final output
            scalar.wait_ge(v_soft, 2)
            scalar.dma_start(out=out[:], in_=o_sb[:]).then_inc(sem_final, 16)
            scalar.wait_ge(sem_final, 16)

    nc.compile()
    return nc


def make_in_maps(dec_hidden, encoder_outputs, W):
    dec_np = np.ascontiguousarray(np.asarray(dec_hidden, dtype=np.float32))
    enc_np = np.ascontiguousarray(np.asarray(encoder_outputs, dtype=np.float32))
    w_np = np.ascontiguousarray(np.asarray(W, dtype=np.float32))
    assert dec_np.shape == (H, B)
    assert enc_np.shape == (T, B, H)
    assert w_np.shape == (H, H)
    in_maps = []
    for c in range(NCORES):
        shard = np.ascontiguousarray(
            enc_np[c * T_L:(c + 1) * T_L].reshape(ROWS, H)
        )
        in_maps.append({"enc": shard, "dec": dec_np, "w": w_np})
    return in_maps


def _install_ntff_hook():
    """The image's antenv lacks axon_hooks; shim it and register the
    ctypes NTFF profile hook so trace=True works under axon."""
    import types

    if "antenv.axon_hooks" in sys.modules:
        return
    import antenv

    mod = types.ModuleType("antenv.axon_hooks")
    state = {"hook": None}
    mod.set_axon_ntff_profile_hook = lambda h: state.__setitem__("hook", h)
    mod.get_axon_ntff_profile_hook = lambda: state["hook"]
    sys.modules["antenv.axon_hooks"] = mod
    antenv.axon_hooks = mod
    try:
        from trn_agent_boot.trn_boot import _ntff_profile_via_ctypes

        mod.set_axon_ntff_profile_hook(
            _ntff_profile_via_ctypes("/opt/axon/libaxon_pjrt.so")
        )
    except Exception as e:  # degrade to no tracing
        print(f"ntff hook install failed: {e}", file=sys.stderr)


def run(dec_hidden, encoder_outputs, W, trace=False):
    if trace:
        _install_ntff_hook()
    nc = build_nc()
    in_maps = make_in_maps(dec_hidden, encoder_outputs, W)
    res = run_bass_kernel_spmd(
        nc, in_maps, core_ids=list(range(NCORES)), trace=trace
    )
    out = np.asarray(res.results[0]["out"], dtype=np.float32)
    return out, res


def kernel(dec_hidden, encoder_outputs, W):
    out, _ = run(dec_hidden, encoder_outputs, W, trace=False)
    return out
